# revision 1
# baseline (speedup 1.0000x reference)
"""Trainium2 Bass kernel for nn_ContrastiveLoss (wav2vec2-style contrastive loss).

Shapes (hardcoded): B=8, C=256, T=1024, M=512 masked positions, K=100 negatives.
Sharding: pure data parallel — batch row b -> NeuronCore b (8 cores).

Per core the dominant work is streaming negatives[b] ([M, K, C] f32, 52.4 MB)
from HBM once, computing per (m, k):
    dot[m,k]   = sum_c neg[m,k,c] * ctx_m[m,c]
    sumsq[m,k] = sum_c neg[m,k,c]^2
then cosine normalization, logsumexp over K+1 logits, per-row loss.

Engine split: VectorE does all dots (fused scalar_tensor_tensor multiply +
accumulate) plus a small share of the sumsq; ScalarE does the rest of the
sumsq (activation Square with accum_out, outputs in PSUM which has lower
per-op overhead for ScalarE). Epilogues for all 4 m-groups run at the end,
grouped by activation function to avoid ACT table reloads.
The device returns per-row losses [128, 4] per core; the host sums & divides.
"""

import numpy as np

TEMP = 0.1
EPS = 1e-8
B, C, T = 8, 256, 1024
M = 512  # masked positions per batch row
K = 100  # negatives per masked position
P = 128  # partitions
G = M // P  # m-groups per core (4)
KCH = 10  # k's per streamed tile: [128, KCH, C] f32 = 1.25 MB
NKC = K // KCH  # stream tiles per m-group (10)
SPLIT = 5  # k % SPLIT == 0 -> sumsq on VectorE, else ScalarE

_NC = None


def _build_nc():
    import concourse.bacc as bacc
    import concourse.tile as tile
    from concourse import mybir

    f32 = mybir.dt.float32
    Alu = mybir.AluOpType
    Act = mybir.ActivationFunctionType

    nc = bacc.Bacc(trn_type="TRN2")
    neg = nc.dram_tensor("neg", [M, K, C], f32, kind="ExternalInput")
    ctxg = nc.dram_tensor("ctxg", [M, C], f32, kind="ExternalInput")
    posg = nc.dram_tensor("posg", [M, C], f32, kind="ExternalInput")
    rowloss = nc.dram_tensor("rowloss", [P, G], f32, kind="ExternalOutput")

    with tile.TileContext(nc) as tc:
        with (
            tc.tile_pool(name="stream", bufs=5) as stream,
            tc.tile_pool(name="grp", bufs=2) as grp,
            tc.tile_pool(name="pg", bufs=G) as pg,
            tc.tile_pool(name="scrp", bufs=2) as scrp,
            tc.tile_pool(name="psg", bufs=G) as psg,
            tc.tile_pool(name="pss", bufs=2) as pss,
            tc.tile_pool(name="outp", bufs=1) as outp,
        ):
            out_t = outp.tile([P, G], f32)
            # per-group persistent tiles (epilogue runs after all streaming)
            gt = {}
            for g in range(G):
                gt[g] = dict(
                    css=pg.tile([P, 1], f32, tag="css", name=f"css{g}"),
                    pss_t=pg.tile([P, 1], f32, tag="pss_t", name=f"pss_t{g}"),
                    cpd=pg.tile([P, 1], f32, tag="cpd", name=f"cpd{g}"),
                    rawdots=pg.tile([P, K], f32, tag="rawdots", name=f"rawdots{g}"),
                    negss=psg.tile([P, K], f32, tag="negss", name=f"negss{g}"),
                    logits=pg.tile([P, K + 1], f32, tag="logits", name=f"logits{g}"),
                )

            for g in range(G):
                m0 = g * P
                d = gt[g]
                ctx_t = grp.tile([P, C], f32, tag="ctx")
                pos_t = grp.tile([P, C], f32, tag="pos")
                nc.sync.dma_start(out=ctx_t[:], in_=ctxg[m0 : m0 + P, :])
                nc.sync.dma_start(out=pos_t[:], in_=posg[m0 : m0 + P, :])

                scr = scrp.tile([P, C], f32, tag="scr")
                nc.vector.scalar_tensor_tensor(
                    out=scr[:], in0=ctx_t[:], scalar=1.0, in1=ctx_t[:],
                    op0=Alu.mult, op1=Alu.mult, accum_out=d["css"][:],
                )
                nc.vector.scalar_tensor_tensor(
                    out=scr[:], in0=pos_t[:], scalar=1.0, in1=pos_t[:],
                    op0=Alu.mult, op1=Alu.mult, accum_out=d["pss_t"][:],
                )
                nc.vector.scalar_tensor_tensor(
                    out=scr[:], in0=ctx_t[:], scalar=1.0, in1=pos_t[:],
                    op0=Alu.mult, op1=Alu.mult, accum_out=d["cpd"][:],
                )

                for t in range(NKC):
                    nt = stream.tile([P, KCH, C], f32, tag="nt")
                    nc.sync.dma_start(
                        out=nt[:],
                        in_=neg[m0 : m0 + P, t * KCH : (t + 1) * KCH, :],
                    )
                    for j in range(KCH):
                        k = t * KCH + j
                        nc.vector.scalar_tensor_tensor(
                            out=scr[:], in0=nt[:, j, :], scalar=1.0, in1=ctx_t[:],
                            op0=Alu.mult, op1=Alu.mult,
                            accum_out=d["rawdots"][:, k : k + 1],
                        )
                        if k % SPLIT == 0:
                            nc.vector.scalar_tensor_tensor(
                                out=scr[:], in0=nt[:, j, :], scalar=1.0,
                                in1=nt[:, j, :], op0=Alu.mult, op1=Alu.mult,
                                accum_out=d["negss"][:, k : k + 1],
                            )
                        else:
                            scr2 = pss.tile([P, C], f32, tag="scr2")
                            nc.scalar.activation(
                                out=scr2[:], in_=nt[:, j, :], func=Act.Square,
                                accum_out=d["negss"][:, k : k + 1],
                            )

            # ---- batched epilogue, grouped by ACT function ----
            crn, prn, nrn, mx, mxs, se, lnse, t1 = {}, {}, {}, {}, {}, {}, {}, {}
            for g in range(G):
                d = gt[g]
                crn[g] = pg.tile([P, 1], f32, tag="crn", name=f"crn{g}")
                prn[g] = pg.tile([P, 1], f32, tag="prn", name=f"prn{g}")
                nrn[g] = pg.tile([P, K], f32, tag="nrn", name=f"nrn{g}")
            # all sqrts first (one Sqrt table load)
            for g in range(G):
                d = gt[g]
                nc.scalar.sqrt(d["css"][:], d["css"][:])
                nc.scalar.sqrt(d["pss_t"][:], d["pss_t"][:])
                nc.scalar.sqrt(d["negss"][:], d["negss"][:])
            for g in range(G):
                d = gt[g]
                nc.vector.tensor_scalar_max(d["css"][:], d["css"][:], EPS)
                nc.vector.tensor_scalar_max(d["pss_t"][:], d["pss_t"][:], EPS)
                nc.vector.tensor_scalar_max(d["negss"][:], d["negss"][:], EPS)
                nc.vector.reciprocal(crn[g][:], d["css"][:])
                nc.vector.reciprocal(prn[g][:], d["pss_t"][:])
                nc.vector.reciprocal(nrn[g][:], d["negss"][:])
                # logits: col 0 = positive sim, cols 1..K = negative sims
                nc.vector.scalar_tensor_tensor(
                    out=d["logits"][:, 0:1], in0=d["cpd"][:], scalar=crn[g][:],
                    in1=prn[g][:], op0=Alu.mult, op1=Alu.mult,
                )
                nc.vector.scalar_tensor_tensor(
                    out=d["logits"][:, 1 : K + 1], in0=d["rawdots"][:],
                    scalar=crn[g][:], in1=nrn[g][:], op0=Alu.mult, op1=Alu.mult,
                )
                mx[g] = pg.tile([P, 1], f32, tag="mx", name=f"mx{g}")
                mxs[g] = pg.tile([P, 1], f32, tag="mxs", name=f"mxs{g}")
                nc.vector.reduce_max(
                    mx[g][:], d["logits"][:], axis=mybir.AxisListType.X
                )
                nc.vector.tensor_scalar_mul(mxs[g][:], mx[g][:], -1.0 / TEMP)
            # all exps (one Exp table load)
            for g in range(G):
                d = gt[g]
                esc = scrp.tile([P, K + 1], f32, tag="esc")
                se[g] = pg.tile([P, 1], f32, tag="se", name=f"se{g}")
                nc.scalar.activation(
                    out=esc[:], in_=d["logits"][:], func=Act.Exp,
                    scale=1.0 / TEMP, bias=mxs[g][:], accum_out=se[g][:],
                )
            # all lns (one Ln table load)
            for g in range(G):
                lnse[g] = pg.tile([P, 1], f32, tag="lnse", name=f"lnse{g}")
                nc.scalar.activation(out=lnse[g][:], in_=se[g][:], func=Act.Ln)
            for g in range(G):
                d = gt[g]
                t1[g] = pg.tile([P, 1], f32, tag="t1", name=f"t1{g}")
                nc.vector.scalar_tensor_tensor(
                    out=t1[g][:], in0=mx[g][:], scalar=1.0 / TEMP, in1=lnse[g][:],
                    op0=Alu.mult, op1=Alu.add,
                )
                nc.vector.scalar_tensor_tensor(
                    out=out_t[:, g : g + 1], in0=d["logits"][:, 0:1],
                    scalar=-1.0 / TEMP, in1=t1[g][:], op0=Alu.mult, op1=Alu.add,
                )
            nc.sync.dma_start(out=rowloss[:], in_=out_t[:])
    nc.finalize()
    return nc


def _get_nc():
    global _NC
    if _NC is None:
        _NC = _build_nc()
    return _NC


def kernel(context, positive, negatives, mask_indices, num_masked):
    from concourse.bass_utils import run_bass_kernel_spmd

    context = np.asarray(context, dtype=np.float32)
    positive = np.asarray(positive, dtype=np.float32)
    negatives = np.asarray(negatives, dtype=np.float32)
    mask = np.asarray(mask_indices).astype(bool)
    nm = int(np.asarray(num_masked))
    assert nm == M, f"kernel hardcodes num_masked={M}, got {nm}"
    assert context.shape == (B, C, T) and negatives.shape == (B, M, K, C)

    in_maps = []
    for b in range(B):
        idx = np.flatnonzero(mask[b])
        assert idx.size == M, f"row {b}: expected {M} masked, got {idx.size}"
        ctxg = np.ascontiguousarray(context[b].T[idx])  # [M, C]
        posg = np.ascontiguousarray(positive[b].T[idx])  # [M, C]
        in_maps.append(
            {
                "neg": np.ascontiguousarray(negatives[b]),
                "ctxg": ctxg,
                "posg": posg,
            }
        )

    res = run_bass_kernel_spmd(_get_nc(), in_maps, core_ids=list(range(B)))
    total = np.float64(0.0)
    for r in res.results:
        total += r["rowloss"].astype(np.float64).sum()
    return np.float32(total / (B * M))



# revision 5
# speedup vs baseline: 1.3554x; 1.3554x over previous
"""Trainium2 Bass kernel for nn_ContrastiveLoss (wav2vec2-style contrastive loss).

Shapes (hardcoded): B=8, C=256, T=1024, M=512 masked positions, K=100 negatives.
Sharding: pure data parallel - batch row b -> NeuronCore b (8 cores).

Strategy: negatives are uploaded as bf16 with each length-256 c-vector padded
to 258 (two zero pad slots). A runtime-registered custom DVE op streams a
[128, KCH*258] tile once and emits, per element, an alternating pair of
running prefix sums: cumsum(neg*ctx) at even positions, cumsum(neg^2) at odd
positions (both fp32 internal). The two pad slots at the end of each chunk
therefore hold the chunk-complete cumsums of both quantities; one strided copy
plus two small subtracts recover per-k dots and sum-of-squares. This computes
BOTH reductions in ~1.008 DVE cycles per streamed element, vs ~2 instructions
(and 2x the cycles) per k with stock scalar_tensor_tensor + activation.

ScalarE handles the small prologue (ctx/pos norms) and the exp/ln/sqrt
epilogue; everything is grouped to avoid ACT table thrash. The device returns
per-row losses [128, 4] per core; the host sums and divides.
"""

import numpy as np

TEMP = 0.1
EPS = 1e-8
B, C, T = 8, 256, 1024
M = 512  # masked positions per batch row
K = 100  # negatives per masked position
P = 128  # partitions
G = M // P  # m-groups per core (4)
NPAD = C + 2  # padded chunk length (256 data + 2 pad)
KCH = 20  # k's per streamed tile
NT = K // KCH  # stream tiles per m-group (5)

_NC = None
_OP = None

OP_NAME = "DUAL_CUMSUM_ANT"


def _register_op():
    """Register the dual-cumsum custom DVE op (idempotent)."""
    global _OP
    if _OP is not None:
        return _OP
    import concourse.dve_ops as dvo
    from concourse.dve_ops import DveOp
    from concourse.dve_spec import Spec, Src0, Src1, Zero, One, select, scan, AluOp, lower
    from concourse.dve_uop import DveOpSpec
    from concourse.dve_table_gen import dve_ver_for

    if OP_NAME in dvo._SUB_OPCODE_FOR_NAME:
        _OP = next(o for o in dvo.OPS if o.name == OP_NAME)
        return _OP

    def _ref(in0, in1, c0, c1, c2):
        Pp = in0.shape[0]
        a = np.asarray(in0, np.float32).reshape(Pp, -1)
        b = np.asarray(in1, np.float32).reshape(Pp, -1)
        prod = np.cumsum(a * b, axis=1)
        sqs = np.cumsum(a * a, axis=1)
        k = np.arange(a.shape[1])
        alt = (k % 2 == 0)  # xor-scan of ones seeded 0: TRUE at even positions
        return np.where(alt[None, :], prod, sqs).reshape(in0.shape)

    s1 = scan(AluOp.ADD, Src0 * Src1)
    s2 = scan(AluOp.ADD, Src0 * Src0)
    alt = scan(AluOp.LOGICAL_XOR, One, init=Zero)
    spec = Spec(body=select(alt, s1, s2), reference=_ref)

    row = max(dvo._SUB_OPCODE_FOR_NAME.values()) + 1
    assert row < 0x20
    dvo._SUB_OPCODE_FOR_NAME[OP_NAME] = row
    ver = dve_ver_for("TRN2")
    uops = lower(spec, ver=ver)
    sha = DveOpSpec(name=OP_NAME, opcode=row, uops=uops, rd1_en=True).sha(ver)
    op = DveOp(OP_NAME, spec, subdim=False, uops_sha={ver: sha})
    dvo.OPS.append(op)
    dvo.CUSTOM_DVE_SPECS[OP_NAME] = spec
    _OP = op
    return op


def _build_nc():
    import concourse.bacc as bacc
    import concourse.tile as tile
    from concourse import mybir

    op = _register_op()

    f32 = mybir.dt.float32
    bf16 = mybir.dt.bfloat16
    Alu = mybir.AluOpType
    Act = mybir.ActivationFunctionType

    nc = bacc.Bacc(trn_type="TRN2")
    neg = nc.dram_tensor("neg", [M, K, NPAD], bf16, kind="ExternalInput")
    ctxp = nc.dram_tensor("ctxp", [M, NPAD], bf16, kind="ExternalInput")
    posg = nc.dram_tensor("posg", [M, C], bf16, kind="ExternalInput")
    rowloss = nc.dram_tensor("rowloss", [P, G], f32, kind="ExternalOutput")

    with tile.TileContext(nc) as tc:
        with (
            tc.tile_pool(name="stream", bufs=3) as stream,
            tc.tile_pool(name="bigp", bufs=2) as bigp,
            tc.tile_pool(name="grp", bufs=2) as grp,
            tc.tile_pool(name="pg", bufs=G) as pg,
            tc.tile_pool(name="scrp", bufs=2) as scrp,
            tc.tile_pool(name="pss", bufs=2) as psp,
            tc.tile_pool(name="outp", bufs=1) as outp,
        ):
            out_t = outp.tile([P, G], f32, tag="out_t")
            # batched per-group scalars: columns = groups
            cps_t = outp.tile([P, 2 * G], f32, tag="cps_t")  # css col g, pss col G+g
            cpd_t = outp.tile([P, G], f32, tag="cpd_t")
            l0_t = outp.tile([P, G], f32, tag="l0_t")
            mx_t = outp.tile([P, G], f32, tag="mx_t")
            mxs_t = outp.tile([P, G], f32, tag="mxs_t")
            se_t = outp.tile([P, G], f32, tag="se_t")
            lnse_t = outp.tile([P, G], f32, tag="lnse_t")
            t1_t = outp.tile([P, G], f32, tag="t1_t")
            cumt = outp.tile([P, KCH + 1, 2], f32, tag="cumt")
            nc.gpsimd.memset(cumt[:, 0:1, :], 0.0)

            gt = {}
            for g in range(G):
                gt[g] = dict(
                    rawdots=pg.tile([P, K], f32, tag="rawdots", name=f"rawdots{g}"),
                    negss=pg.tile([P, K], f32, tag="negss", name=f"negss{g}"),
                    logits=pg.tile([P, K + 1], f32, tag="logits", name=f"logits{g}"),
                    nrn=pg.tile([P, K], f32, tag="nrn", name=f"nrn{g}"),
                )

            for g in range(G):
                m0 = g * P
                d = gt[g]
                ctx_t = grp.tile([P, NPAD], bf16, tag="ctx")
                pos_t = grp.tile([P, C], bf16, tag="pos")
                nc.sync.dma_start(out=ctx_t[:], in_=ctxp[m0 : m0 + P, :])
                nc.sync.dma_start(out=pos_t[:], in_=posg[m0 : m0 + P, :])

                # prologue: ctx/pos self-norms on ScalarE, ctx.pos dot on DVE
                sq_ps = psp.tile([P, C], f32, tag="sq_ps")
                nc.scalar.activation(
                    out=sq_ps[:], in_=ctx_t[:, 0:C], func=Act.Square,
                    accum_out=cps_t[:, g : g + 1],
                )
                sq_ps2 = psp.tile([P, C], f32, tag="sq_ps2")
                nc.scalar.activation(
                    out=sq_ps2[:], in_=pos_t[:], func=Act.Square,
                    accum_out=cps_t[:, G + g : G + g + 1],
                )
                scr = scrp.tile([P, C], bf16, tag="scr")
                nc.vector.scalar_tensor_tensor(
                    out=scr[:], in0=ctx_t[:, 0:C], scalar=1.0, in1=pos_t[:],
                    op0=Alu.mult, op1=Alu.mult, accum_out=cpd_t[:, g : g + 1],
                )

                ctx_bc = ctx_t[:].unsqueeze(1).broadcast_to([P, KCH, NPAD])
                for t in range(NT):
                    nt = stream.tile([P, KCH, NPAD], bf16, tag="nt")
                    nc.sync.dma_start(
                        out=nt[:],
                        in_=neg[m0 : m0 + P, t * KCH : (t + 1) * KCH, :],
                    )
                    big = bigp.tile([P, KCH * NPAD], f32, tag="big")
                    nc.vector._custom_dve(
                        op, out=big[:], in0=nt[:], in1=ctx_bc, s0=0.0, s1=0.0
                    )
                    big3 = big[:].rearrange("p (s n) -> p s n", s=KCH)
                    nc.vector.tensor_copy(
                        cumt[:, 1 : KCH + 1, :], big3[:, :, C : C + 2]
                    )
                    k0 = t * KCH
                    # dot cumsum sits at even pad slot (C), sq cumsum at C+1
                    nc.vector.tensor_sub(
                        d["rawdots"][:, k0 : k0 + KCH],
                        cumt[:, 1 : KCH + 1, 0],
                        cumt[:, 0:KCH, 0],
                    )
                    nc.vector.tensor_sub(
                        d["negss"][:, k0 : k0 + KCH],
                        cumt[:, 1 : KCH + 1, 1],
                        cumt[:, 0:KCH, 1],
                    )

            # ---- epilogue (batched across groups, ACT funcs grouped) ----
            # sqrt of all norms (one Sqrt table load)
            nc.scalar.sqrt(cps_t[:], cps_t[:])
            for g in range(G):
                d = gt[g]
                nc.scalar.sqrt(d["negss"][:], d["negss"][:])
            nc.vector.tensor_scalar_max(cps_t[:], cps_t[:], EPS)
            nc.vector.reciprocal_approx_fast(cps_t[:], cps_t[:])
            for g in range(G):
                d = gt[g]
                nc.vector.tensor_scalar_max(d["negss"][:], d["negss"][:], EPS)
                nc.vector.reciprocal_approx_fast(d["nrn"][:], d["negss"][:])
                # logits col 0 = cpd * crn * prn ; cols 1..K = rawdots * crn * nrn
                nc.vector.scalar_tensor_tensor(
                    out=d["logits"][:, 0:1], in0=cpd_t[:, g : g + 1],
                    scalar=cps_t[:, g : g + 1], in1=cps_t[:, G + g : G + g + 1],
                    op0=Alu.mult, op1=Alu.mult,
                )
                nc.vector.tensor_copy(l0_t[:, g : g + 1], d["logits"][:, 0:1])
                nc.vector.scalar_tensor_tensor(
                    out=d["logits"][:, 1 : K + 1], in0=d["rawdots"][:],
                    scalar=cps_t[:, g : g + 1], in1=d["nrn"][:],
                    op0=Alu.mult, op1=Alu.mult,
                )
                nc.vector.reduce_max(
                    mx_t[:, g : g + 1], d["logits"][:], axis=mybir.AxisListType.X
                )
            nc.vector.tensor_scalar_mul(mxs_t[:], mx_t[:], -1.0 / TEMP)
            # exps (one Exp table load)
            for g in range(G):
                d = gt[g]
                esc = scrp.tile([P, K + 1], f32, tag="esc")
                nc.scalar.activation(
                    out=esc[:], in_=d["logits"][:], func=Act.Exp,
                    scale=1.0 / TEMP, bias=mxs_t[:, g : g + 1],
                    accum_out=se_t[:, g : g + 1],
                )
            nc.scalar.activation(out=lnse_t[:], in_=se_t[:], func=Act.Ln)
            nc.vector.scalar_tensor_tensor(
                out=t1_t[:], in0=mx_t[:], scalar=1.0 / TEMP, in1=lnse_t[:],
                op0=Alu.mult, op1=Alu.add,
            )
            nc.vector.scalar_tensor_tensor(
                out=out_t[:], in0=l0_t[:], scalar=-1.0 / TEMP, in1=t1_t[:],
                op0=Alu.mult, op1=Alu.add,
            )
            nc.sync.dma_start(out=rowloss[:], in_=out_t[:])
    nc.finalize()
    return nc


def _get_nc():
    global _NC
    if _NC is None:
        _NC = _build_nc()
    return _NC


def make_in_maps(context, positive, negatives, mask_indices):
    import ml_dtypes

    bf = ml_dtypes.bfloat16
    context = np.asarray(context, dtype=np.float32)
    positive = np.asarray(positive, dtype=np.float32)
    negatives = np.asarray(negatives, dtype=np.float32)
    mask = np.asarray(mask_indices).astype(bool)

    in_maps = []
    for b in range(B):
        idx = np.flatnonzero(mask[b])
        assert idx.size == M, f"row {b}: expected {M} masked, got {idx.size}"
        ctxg = context[b].T[idx]  # [M, C] f32
        posg = positive[b].T[idx]
        ctxp = np.zeros((M, NPAD), dtype=bf)
        ctxp[:, :C] = ctxg.astype(bf)
        negp = np.zeros((M, K, NPAD), dtype=bf)
        negp[:, :, :C] = negatives[b].astype(bf)
        in_maps.append(
            {
                "neg": negp,
                "ctxp": ctxp,
                "posg": np.ascontiguousarray(posg.astype(bf)),
            }
        )
    return in_maps


def kernel(context, positive, negatives, mask_indices, num_masked):
    from concourse.bass_utils import run_bass_kernel_spmd

    nm = int(np.asarray(num_masked))
    assert nm == M, f"kernel hardcodes num_masked={M}, got {nm}"
    assert np.asarray(context).shape == (B, C, T)
    assert np.asarray(negatives).shape == (B, M, K, C)

    in_maps = make_in_maps(context, positive, negatives, mask_indices)
    res = run_bass_kernel_spmd(_get_nc(), in_maps, core_ids=list(range(B)))
    total = np.float64(0.0)
    for r in res.results:
        total += r["rowloss"].astype(np.float64).sum()
    return np.float32(total / (B * M))


# revision 6
# speedup vs baseline: 1.4603x; 1.0774x over previous
"""Trainium2 Bass kernel for nn_ContrastiveLoss (wav2vec2-style contrastive loss).

Shapes (hardcoded): B=8, C=256, T=1024, M=512 masked positions, K=100 negatives.
Sharding: pure data parallel - batch row b -> NeuronCore b (8 cores).

Strategy: negatives are uploaded as bf16 with each length-256 c-vector padded
to 258 (two zero pad slots). A runtime-registered custom DVE op streams a
[128, kch*258] tile once and emits, per element, an alternating pair of
running prefix sums: cumsum(neg*ctx) at even positions, cumsum(neg^2) at odd
positions (fp32 internal). The two pad slots at the end of each chunk hold the
chunk-complete cumsums of both quantities; a strided copy plus two subtracts
(on GpSimd, off the critical engine) recover per-k dots and sums of squares.
This computes BOTH reductions at ~1.008 DVE cycles per streamed element.

VectorE runs only the scan ops (+ tiny logit math); ScalarE does sqrt/exp/ln;
GpSimd does extraction/clamps. Small DMAs ride the ACT HWDGE queue so the SP
queue only carries the bulk negative stream. Group 0 starts with small tiles
so the first scan begins as early as possible; per-group epilogues are
interleaved so only group 3's tail is serialized. The device returns per-row
losses [128, 4] per core; the host sums and divides.
"""

import numpy as np

TEMP = 0.1
EPS = 1e-8
B, C, T = 8, 256, 1024
M = 512  # masked positions per batch row
K = 100  # negatives per masked position
P = 128  # partitions
G = M // P  # m-groups per core (4)
NPAD = C + 2  # padded chunk length (256 data + 2 pad)
KCH = 20  # max k's per streamed tile

# per-group tile splits (k0, kch); group 0 ramps up for an early first scan
_TILES0 = [(0, 4), (4, 8), (12, 8), (20, 20), (40, 20), (60, 20), (80, 20)]
_TILESN = [(k0, KCH) for k0 in range(0, K, KCH)]

_NC = None
_OP = None

OP_NAME = "DUAL_CUMSUM_ANT"


def _register_op():
    """Register the dual-cumsum custom DVE op (idempotent)."""
    global _OP
    if _OP is not None:
        return _OP
    import concourse.dve_ops as dvo
    from concourse.dve_ops import DveOp
    from concourse.dve_spec import Spec, Src0, Src1, Zero, One, select, scan, AluOp, lower
    from concourse.dve_uop import DveOpSpec
    from concourse.dve_table_gen import dve_ver_for

    if OP_NAME in dvo._SUB_OPCODE_FOR_NAME:
        _OP = next(o for o in dvo.OPS if o.name == OP_NAME)
        return _OP

    def _ref(in0, in1, c0, c1, c2):
        Pp = in0.shape[0]
        a = np.asarray(in0, np.float32).reshape(Pp, -1)
        b = np.asarray(in1, np.float32).reshape(Pp, -1)
        prod = np.cumsum(a * b, axis=1)
        sqs = np.cumsum(a * a, axis=1)
        k = np.arange(a.shape[1])
        alt = (k % 2 == 0)  # xor-scan of ones seeded 0: TRUE at even positions
        return np.where(alt[None, :], prod, sqs).reshape(in0.shape)

    s1 = scan(AluOp.ADD, Src0 * Src1)
    s2 = scan(AluOp.ADD, Src0 * Src0)
    alt = scan(AluOp.LOGICAL_XOR, One, init=Zero)
    spec = Spec(body=select(alt, s1, s2), reference=_ref)

    row = max(dvo._SUB_OPCODE_FOR_NAME.values()) + 1
    assert row < 0x20
    dvo._SUB_OPCODE_FOR_NAME[OP_NAME] = row
    ver = dve_ver_for("TRN2")
    uops = lower(spec, ver=ver)
    sha = DveOpSpec(name=OP_NAME, opcode=row, uops=uops, rd1_en=True).sha(ver)
    op = DveOp(OP_NAME, spec, subdim=False, uops_sha={ver: sha})
    dvo.OPS.append(op)
    dvo.CUSTOM_DVE_SPECS[OP_NAME] = spec
    _OP = op
    return op


def _build_nc():
    import concourse.bacc as bacc
    import concourse.tile as tile
    from concourse import mybir

    op = _register_op()

    f32 = mybir.dt.float32
    bf16 = mybir.dt.bfloat16
    Alu = mybir.AluOpType
    Act = mybir.ActivationFunctionType
    EPS2 = EPS * EPS

    nc = bacc.Bacc(trn_type="TRN2")
    neg = nc.dram_tensor("neg", [M, K, NPAD], bf16, kind="ExternalInput")
    ctxp = nc.dram_tensor("ctxp", [M, NPAD], bf16, kind="ExternalInput")
    posg = nc.dram_tensor("posg", [M, C], bf16, kind="ExternalInput")
    rowloss = nc.dram_tensor("rowloss", [P, G], f32, kind="ExternalOutput")

    with tile.TileContext(nc) as tc:
        with (
            tc.tile_pool(name="stream", bufs=4) as stream,
            tc.tile_pool(name="bigp", bufs=3) as bigp,
            tc.tile_pool(name="grp", bufs=G) as grp,
            tc.tile_pool(name="pg", bufs=G) as pg,
            tc.tile_pool(name="scrp", bufs=2) as scrp,
            tc.tile_pool(name="outp", bufs=1) as outp,
        ):
            out_t = outp.tile([P, G], f32, tag="out_t")
            cps_t = outp.tile([P, 2 * G], f32, tag="cps_t")  # css col g, pss col G+g
            cpd_t = outp.tile([P, G], f32, tag="cpd_t")
            l0_t = outp.tile([P, G], f32, tag="l0_t")
            mx_t = outp.tile([P, G], f32, tag="mx_t")
            mxs_t = outp.tile([P, G], f32, tag="mxs_t")
            se_t = outp.tile([P, G], f32, tag="se_t")
            lnse_t = outp.tile([P, G], f32, tag="lnse_t")
            t1_t = outp.tile([P, G], f32, tag="t1_t")
            cumt = outp.tile([P, KCH + 1, 2], f32, tag="cumt")
            nc.gpsimd.memset(cumt[:, 0:1, :], 0.0)

            gt = {}
            for g in range(G):
                gt[g] = dict(
                    rawdots=pg.tile([P, K], f32, tag="rawdots", name=f"rawdots{g}"),
                    negss=pg.tile([P, K], f32, tag="negss", name=f"negss{g}"),
                    logits=pg.tile([P, K + 1], f32, tag="logits", name=f"logits{g}"),
                    nrn=pg.tile([P, K], f32, tag="nrn", name=f"nrn{g}"),
                )

            # --- up-front small DMAs (ACT HWDGE queue) + prologue compute ---
            ctx_ts, pos_ts = {}, {}
            for g in range(G):
                m0 = g * P
                ctx_ts[g] = grp.tile([P, NPAD], bf16, tag="ctx", name=f"ctx{g}")
                nc.scalar.dma_start(out=ctx_ts[g][:], in_=ctxp[m0 : m0 + P, :])
            for g in range(G):
                m0 = g * P
                pos_ts[g] = grp.tile([P, C], bf16, tag="pos", name=f"pos{g}")
                nc.scalar.dma_start(out=pos_ts[g][:], in_=posg[m0 : m0 + P, :])
            for g in range(G):
                sq_s = scrp.tile([P, C], f32, tag="sq_s")
                nc.scalar.activation(
                    out=sq_s[:], in_=ctx_ts[g][:, 0:C], func=Act.Square,
                    accum_out=cps_t[:, g : g + 1],
                )
                sq_s2 = scrp.tile([P, C], f32, tag="sq_s2")
                nc.scalar.activation(
                    out=sq_s2[:], in_=pos_ts[g][:], func=Act.Square,
                    accum_out=cps_t[:, G + g : G + g + 1],
                )
                scr = scrp.tile([P, C], bf16, tag="scr")
                nc.vector.scalar_tensor_tensor(
                    out=scr[:], in0=ctx_ts[g][:, 0:C], scalar=1.0, in1=pos_ts[g][:],
                    op0=Alu.mult, op1=Alu.mult, accum_out=cpd_t[:, g : g + 1],
                )
            # crn/prn = 1/max(sqrt(ss), EPS) computed as 1/sqrt(max(ss, EPS^2))
            nc.gpsimd.tensor_scalar_max(cps_t[:], cps_t[:], EPS2)
            nc.scalar.sqrt(cps_t[:], cps_t[:])
            nc.vector.reciprocal_approx_fast(cps_t[:], cps_t[:])

            def epilogue(g):
                d = gt[g]
                nc.gpsimd.tensor_scalar_max(d["negss"][:], d["negss"][:], EPS2)
                nc.scalar.sqrt(d["negss"][:], d["negss"][:])
                nc.vector.reciprocal_approx_fast(d["nrn"][:], d["negss"][:])
                nc.vector.scalar_tensor_tensor(
                    out=d["logits"][:, 0:1], in0=cpd_t[:, g : g + 1],
                    scalar=cps_t[:, g : g + 1], in1=cps_t[:, G + g : G + g + 1],
                    op0=Alu.mult, op1=Alu.mult,
                )
                nc.gpsimd.tensor_copy(l0_t[:, g : g + 1], d["logits"][:, 0:1])
                nc.vector.scalar_tensor_tensor(
                    out=d["logits"][:, 1 : K + 1], in0=d["rawdots"][:],
                    scalar=cps_t[:, g : g + 1], in1=d["nrn"][:],
                    op0=Alu.mult, op1=Alu.mult,
                )
                nc.vector.reduce_max(
                    mx_t[:, g : g + 1], d["logits"][:], axis=mybir.AxisListType.X
                )
                nc.gpsimd.tensor_scalar_mul(
                    mxs_t[:, g : g + 1], mx_t[:, g : g + 1], -1.0 / TEMP
                )
                esc = scrp.tile([P, K + 1], f32, tag="esc")
                nc.scalar.activation(
                    out=esc[:], in_=d["logits"][:], func=Act.Exp,
                    scale=1.0 / TEMP, bias=mxs_t[:, g : g + 1],
                    accum_out=se_t[:, g : g + 1],
                )

            # --- main streaming loop, epilogue(g-1) deferred into group g ---
            for g in range(G):
                m0 = g * P
                d = gt[g]
                ctx_t = ctx_ts[g]
                tiles = _TILES0 if g == 0 else _TILESN
                for i, (k0, kch) in enumerate(tiles):
                    nt = stream.tile([P, kch, NPAD], bf16, tag="nt")
                    nc.sync.dma_start(
                        out=nt[:], in_=neg[m0 : m0 + P, k0 : k0 + kch, :]
                    )
                    big = bigp.tile([P, kch * NPAD], f32, tag="big")
                    ctx_bc = ctx_t[:].unsqueeze(1).broadcast_to([P, kch, NPAD])
                    nc.vector._custom_dve(
                        op, out=big[:], in0=nt[:], in1=ctx_bc, s0=0.0, s1=0.0
                    )
                    big3 = big[:].rearrange("p (s n) -> p s n", s=kch)
                    nc.gpsimd.tensor_copy(
                        cumt[:, 1 : kch + 1, :], big3[:, :, C : C + 2]
                    )
                    # dot cumsum sits at even pad slot (C), sq cumsum at C+1
                    nc.gpsimd.tensor_sub(
                        d["rawdots"][:, k0 : k0 + kch],
                        cumt[:, 1 : kch + 1, 0],
                        cumt[:, 0:kch, 0],
                    )
                    nc.gpsimd.tensor_sub(
                        d["negss"][:, k0 : k0 + kch],
                        cumt[:, 1 : kch + 1, 1],
                        cumt[:, 0:kch, 1],
                    )
                    if i == 1 and g > 0:
                        epilogue(g - 1)
            epilogue(G - 1)

            # --- tail ---
            nc.scalar.activation(out=lnse_t[:], in_=se_t[:], func=Act.Ln)
            nc.vector.scalar_tensor_tensor(
                out=t1_t[:], in0=mx_t[:], scalar=1.0 / TEMP, in1=lnse_t[:],
                op0=Alu.mult, op1=Alu.add,
            )
            nc.vector.scalar_tensor_tensor(
                out=out_t[:], in0=l0_t[:], scalar=-1.0 / TEMP, in1=t1_t[:],
                op0=Alu.mult, op1=Alu.add,
            )
            nc.scalar.dma_start(out=rowloss[:], in_=out_t[:])
    nc.finalize()
    return nc


def _get_nc():
    global _NC
    if _NC is None:
        _NC = _build_nc()
    return _NC


def make_in_maps(context, positive, negatives, mask_indices):
    import ml_dtypes

    bf = ml_dtypes.bfloat16
    context = np.asarray(context, dtype=np.float32)
    positive = np.asarray(positive, dtype=np.float32)
    negatives = np.asarray(negatives, dtype=np.float32)
    mask = np.asarray(mask_indices).astype(bool)

    in_maps = []
    for b in range(B):
        idx = np.flatnonzero(mask[b])
        assert idx.size == M, f"row {b}: expected {M} masked, got {idx.size}"
        ctxg = context[b].T[idx]  # [M, C] f32
        posg = positive[b].T[idx]
        ctxp = np.zeros((M, NPAD), dtype=bf)
        ctxp[:, :C] = ctxg.astype(bf)
        negp = np.zeros((M, K, NPAD), dtype=bf)
        negp[:, :, :C] = negatives[b].astype(bf)
        in_maps.append(
            {
                "neg": negp,
                "ctxp": ctxp,
                "posg": np.ascontiguousarray(posg.astype(bf)),
            }
        )
    return in_maps


def kernel(context, positive, negatives, mask_indices, num_masked):
    from concourse.bass_utils import run_bass_kernel_spmd

    nm = int(np.asarray(num_masked))
    assert nm == M, f"kernel hardcodes num_masked={M}, got {nm}"
    assert np.asarray(context).shape == (B, C, T)
    assert np.asarray(negatives).shape == (B, M, K, C)

    in_maps = make_in_maps(context, positive, negatives, mask_indices)
    res = run_bass_kernel_spmd(_get_nc(), in_maps, core_ids=list(range(B)))
    total = np.float64(0.0)
    for r in res.results:
        total += r["rowloss"].astype(np.float64).sum()
    return np.float32(total / (B * M))


# revision 11
# speedup vs baseline: 1.6399x; 1.1230x over previous
"""Trainium2 Bass kernel for nn_ContrastiveLoss (wav2vec2-style contrastive loss).

Shapes (hardcoded): B=8, C=256, T=1024, M=512 masked positions, K=100 negatives.
Sharding: pure data parallel - batch row b -> NeuronCore b (8 cores).

Strategy: negatives are uploaded as bf16 with each length-256 c-vector padded
to 258 (two zero pad slots). A runtime-registered custom DVE op streams a
[128, kch*258] tile once and emits, per element, an alternating pair of
running prefix sums: cumsum(neg*ctx) at even positions, cumsum(neg^2) at odd
positions (fp32 internal). The two pad slots at the end of each chunk hold the
chunk-complete cumsums of both quantities; a strided copy plus two subtracts
(on GpSimd, off the critical engine) recover per-k dots and sums of squares.
This computes BOTH reductions at ~1.008 DVE cycles per streamed element.

VectorE runs only the scan ops (+ tiny logit math); ScalarE does sqrt/exp/ln;
GpSimd does extraction/clamps. Small DMAs ride the ACT HWDGE queue so the SP
queue only carries the bulk negative stream. Group 0 starts with small tiles
so the first scan begins as early as possible; per-group epilogues are
interleaved so only group 3's tail is serialized. The device returns per-row
losses [128, 4] per core; the host sums and divides.
"""

import numpy as np

TEMP = 0.1
EPS = 1e-8
B, C, T = 8, 256, 1024
M = 512  # masked positions per batch row
K = 100  # negatives per masked position
P = 128  # partitions
G = M // P  # m-groups per core (4)
NPAD = C + 2  # padded chunk length (256 data + 2 pad)
KCH = 20  # max k's per streamed tile

# per-group tile splits (k0, kch); group 0 ramps up for an early first scan,
# growth rate matched to DMA vs scan speed (~1.25x)
_R0 = [4, 5, 6, 8, 10, 12, 15, 20, 20]
_TILES0 = list(zip(np.cumsum([0] + _R0[:-1]).tolist(), _R0))
_TILESN = [(k0, KCH) for k0 in range(0, K, KCH)]

_NC = None
_OP = None

OP_NAME = "DUAL_CUMSUM_ANT"


def _register_op():
    """Register the dual-cumsum custom DVE op (idempotent)."""
    global _OP
    if _OP is not None:
        return _OP
    import concourse.dve_ops as dvo
    from concourse.dve_ops import DveOp
    from concourse.dve_spec import Spec, Src0, Src1, Zero, One, select, scan, AluOp, lower
    from concourse.dve_uop import DveOpSpec
    from concourse.dve_table_gen import dve_ver_for

    if OP_NAME in dvo._SUB_OPCODE_FOR_NAME:
        _OP = next(o for o in dvo.OPS if o.name == OP_NAME)
        return _OP

    def _ref(in0, in1, c0, c1, c2):
        Pp = in0.shape[0]
        a = np.asarray(in0, np.float32).reshape(Pp, -1)
        b = np.asarray(in1, np.float32).reshape(Pp, -1)
        prod = np.cumsum(a * b, axis=1)
        sqs = np.cumsum(a * a, axis=1)
        k = np.arange(a.shape[1])
        alt = (k % 2 == 0)  # xor-scan of ones seeded 0: TRUE at even positions
        return np.where(alt[None, :], prod, sqs).reshape(in0.shape)

    s1 = scan(AluOp.ADD, Src0 * Src1)
    s2 = scan(AluOp.ADD, Src0 * Src0)
    alt = scan(AluOp.LOGICAL_XOR, One, init=Zero)
    spec = Spec(body=select(alt, s1, s2), reference=_ref)

    row = max(dvo._SUB_OPCODE_FOR_NAME.values()) + 1
    assert row < 0x20
    dvo._SUB_OPCODE_FOR_NAME[OP_NAME] = row
    ver = dve_ver_for("TRN2")
    uops = lower(spec, ver=ver)
    sha = DveOpSpec(name=OP_NAME, opcode=row, uops=uops, rd1_en=True).sha(ver)
    op = DveOp(OP_NAME, spec, subdim=False, uops_sha={ver: sha})
    dvo.OPS.append(op)
    dvo.CUSTOM_DVE_SPECS[OP_NAME] = spec
    _OP = op
    return op


def _build_nc():
    import concourse.bacc as bacc
    import concourse.tile as tile
    from concourse import mybir

    op = _register_op()

    f32 = mybir.dt.float32
    bf16 = mybir.dt.bfloat16
    Alu = mybir.AluOpType
    Act = mybir.ActivationFunctionType
    EPS2 = EPS * EPS

    nc = bacc.Bacc(trn_type="TRN2")
    neg = nc.dram_tensor("neg", [M, K, NPAD], bf16, kind="ExternalInput")
    ctxp = nc.dram_tensor("ctxp", [M, NPAD], bf16, kind="ExternalInput")
    posg = nc.dram_tensor("posg", [M, C], bf16, kind="ExternalInput")
    rowloss = nc.dram_tensor("rowloss", [P, G], f32, kind="ExternalOutput")

    with tile.TileContext(nc) as tc:
        with (
            tc.tile_pool(name="stream", bufs=4) as stream,
            tc.tile_pool(name="bigp", bufs=3) as bigp,
            tc.tile_pool(name="grp", bufs=G) as grp,
            tc.tile_pool(name="pg", bufs=G) as pg,
            tc.tile_pool(name="scrp", bufs=2) as scrp,
            tc.tile_pool(name="outp", bufs=1) as outp,
        ):
            out_t = outp.tile([P, G], f32, tag="out_t")
            cps_t = outp.tile([P, 2 * G], f32, tag="cps_t")  # css col g, pss col G+g
            cpd_t = outp.tile([P, G], f32, tag="cpd_t")
            l0_t = outp.tile([P, G], f32, tag="l0_t")
            mx_t = outp.tile([P, G], f32, tag="mx_t")
            mxs_t = outp.tile([P, G], f32, tag="mxs_t")
            se_t = outp.tile([P, G], f32, tag="se_t")
            lnse_t = outp.tile([P, G], f32, tag="lnse_t")
            t1_t = outp.tile([P, G], f32, tag="t1_t")
            cumt0 = outp.tile([P, KCH + 1, 2], f32, tag="cumt0")
            cumt1 = outp.tile([P, KCH + 1, 2], f32, tag="cumt1")
            cumts = [cumt0, cumt1]
            nc.gpsimd.memset(cumt0[:, 0:1, :], 0.0)
            nc.gpsimd.memset(cumt1[:, 0:1, :], 0.0)

            gt = {}
            for g in range(G):
                gt[g] = dict(
                    rawdots=pg.tile([P, K], f32, tag="rawdots", name=f"rawdots{g}"),
                    negss=pg.tile([P, K], f32, tag="negss", name=f"negss{g}"),
                    logits=pg.tile([P, K + 1], f32, tag="logits", name=f"logits{g}"),
                    nrn=pg.tile([P, K], f32, tag="nrn", name=f"nrn{g}"),
                )

            # --- up-front small DMAs (ACT HWDGE queue) + prologue compute ---
            ctx_ts, pos_ts = {}, {}
            for g in range(G):
                m0 = g * P
                ctx_ts[g] = grp.tile([P, NPAD], bf16, tag="ctx", name=f"ctx{g}")
                nc.scalar.dma_start(out=ctx_ts[g][:], in_=ctxp[m0 : m0 + P, :])
            for g in range(G):
                m0 = g * P
                pos_ts[g] = grp.tile([P, C], bf16, tag="pos", name=f"pos{g}")
                nc.scalar.dma_start(out=pos_ts[g][:], in_=posg[m0 : m0 + P, :])
            for g in range(G):
                sq_s = scrp.tile([P, C], f32, tag="sq_s")
                nc.scalar.activation(
                    out=sq_s[:], in_=ctx_ts[g][:, 0:C], func=Act.Square,
                    accum_out=cps_t[:, g : g + 1],
                )
                sq_s2 = scrp.tile([P, C], f32, tag="sq_s2")
                nc.scalar.activation(
                    out=sq_s2[:], in_=pos_ts[g][:], func=Act.Square,
                    accum_out=cps_t[:, G + g : G + g + 1],
                )
                scr = scrp.tile([P, C], bf16, tag="scr")
                nc.vector.scalar_tensor_tensor(
                    out=scr[:], in0=ctx_ts[g][:, 0:C], scalar=1.0, in1=pos_ts[g][:],
                    op0=Alu.mult, op1=Alu.mult, accum_out=cpd_t[:, g : g + 1],
                )
            # crn/prn = 1/max(sqrt(ss), EPS) computed as 1/sqrt(max(ss, EPS^2))
            nc.vector.tensor_scalar_max(cps_t[:], cps_t[:], EPS2)
            nc.scalar.sqrt(cps_t[:], cps_t[:])
            nc.vector.reciprocal_approx_fast(cps_t[:], cps_t[:])

            def epilogue(g):
                d = gt[g]
                nc.vector.tensor_scalar_max(d["negss"][:], d["negss"][:], EPS2)
                nc.scalar.sqrt(d["negss"][:], d["negss"][:])
                nc.vector.reciprocal_approx_fast(d["nrn"][:], d["negss"][:])
                nc.vector.scalar_tensor_tensor(
                    out=d["logits"][:, 0:1], in0=cpd_t[:, g : g + 1],
                    scalar=cps_t[:, g : g + 1], in1=cps_t[:, G + g : G + g + 1],
                    op0=Alu.mult, op1=Alu.mult,
                )
                nc.vector.tensor_copy(l0_t[:, g : g + 1], d["logits"][:, 0:1])
                nc.vector.scalar_tensor_tensor(
                    out=d["logits"][:, 1 : K + 1], in0=d["rawdots"][:],
                    scalar=cps_t[:, g : g + 1], in1=d["nrn"][:],
                    op0=Alu.mult, op1=Alu.mult,
                )
                nc.vector.reduce_max(
                    mx_t[:, g : g + 1], d["logits"][:], axis=mybir.AxisListType.X
                )
                nc.vector.tensor_scalar_mul(
                    mxs_t[:, g : g + 1], mx_t[:, g : g + 1], -1.0 / TEMP
                )
                esc = scrp.tile([P, K + 1], f32, tag="esc")
                nc.scalar.activation(
                    out=esc[:], in_=d["logits"][:], func=Act.Exp,
                    scale=1.0 / TEMP, bias=mxs_t[:, g : g + 1],
                    accum_out=se_t[:, g : g + 1],
                )
                if g == G - 2:
                    # prewarm the Sqrt ACT table so group G-1's tail chain
                    # skips one table load
                    dum = scrp.tile([P, 1], f32, tag="dum")
                    nc.scalar.sqrt(dum[:], se_t[:, 0:1])

            # --- main streaming loop, epilogue(g-1) deferred into group g ---
            for g in range(G):
                m0 = g * P
                d = gt[g]
                ctx_t = ctx_ts[g]
                tiles = _TILES0 if g == 0 else _TILESN
                for i, (k0, kch) in enumerate(tiles):
                    nt = stream.tile([P, kch, NPAD], bf16, tag="nt")
                    nc.sync.dma_start(
                        out=nt[:], in_=neg[m0 : m0 + P, k0 : k0 + kch, :]
                    )
                    big = bigp.tile([P, kch * NPAD], f32, tag="big")
                    ctx_bc = ctx_t[:].unsqueeze(1).broadcast_to([P, kch, NPAD])
                    nc.vector._custom_dve(
                        op, out=big[:], in0=nt[:], in1=ctx_bc, s0=0.0, s1=0.0
                    )
                    big3 = big[:].rearrange("p (s n) -> p s n", s=kch)
                    cumt = cumts[i % 2]
                    nc.scalar.copy(cumt[:, 1 : kch + 1, :], big3[:, :, C : C + 2])
                    # dot cumsum sits at even pad slot (C), sq cumsum at C+1;
                    # alternate the subtract engine: GpSimd takes every other
                    # tile (it is slow per op but otherwise idle); last tile of
                    # each group stays on DVE so the epilogue chain never
                    # waits on GpSimd
                    last = i == len(tiles) - 1
                    eng = nc.gpsimd if (i % 2 == 0 and not last) else nc.vector
                    eng.tensor_sub(
                        d["rawdots"][:, k0 : k0 + kch],
                        cumt[:, 1 : kch + 1, 0],
                        cumt[:, 0:kch, 0],
                    )
                    eng.tensor_sub(
                        d["negss"][:, k0 : k0 + kch],
                        cumt[:, 1 : kch + 1, 1],
                        cumt[:, 0:kch, 1],
                    )
                    if i == 1 and g > 0:
                        epilogue(g - 1)
            epilogue(G - 1)

            # --- tail ---
            nc.scalar.activation(out=lnse_t[:], in_=se_t[:], func=Act.Ln)
            nc.vector.scalar_tensor_tensor(
                out=t1_t[:], in0=mx_t[:], scalar=1.0 / TEMP, in1=lnse_t[:],
                op0=Alu.mult, op1=Alu.add,
            )
            nc.vector.scalar_tensor_tensor(
                out=out_t[:], in0=l0_t[:], scalar=-1.0 / TEMP, in1=t1_t[:],
                op0=Alu.mult, op1=Alu.add,
            )
            nc.scalar.dma_start(out=rowloss[:], in_=out_t[:])
    nc.finalize()
    return nc


def _get_nc():
    global _NC
    if _NC is None:
        _NC = _build_nc()
    return _NC


def make_in_maps(context, positive, negatives, mask_indices):
    import ml_dtypes

    bf = ml_dtypes.bfloat16
    context = np.asarray(context, dtype=np.float32)
    positive = np.asarray(positive, dtype=np.float32)
    negatives = np.asarray(negatives, dtype=np.float32)
    mask = np.asarray(mask_indices).astype(bool)

    in_maps = []
    for b in range(B):
        idx = np.flatnonzero(mask[b])
        assert idx.size == M, f"row {b}: expected {M} masked, got {idx.size}"
        ctxg = context[b].T[idx]  # [M, C] f32
        posg = positive[b].T[idx]
        ctxp = np.zeros((M, NPAD), dtype=bf)
        ctxp[:, :C] = ctxg.astype(bf)
        negp = np.zeros((M, K, NPAD), dtype=bf)
        negp[:, :, :C] = negatives[b].astype(bf)
        in_maps.append(
            {
                "neg": negp,
                "ctxp": ctxp,
                "posg": np.ascontiguousarray(posg.astype(bf)),
            }
        )
    return in_maps


def kernel(context, positive, negatives, mask_indices, num_masked):
    from concourse.bass_utils import run_bass_kernel_spmd

    nm = int(np.asarray(num_masked))
    assert nm == M, f"kernel hardcodes num_masked={M}, got {nm}"
    assert np.asarray(context).shape == (B, C, T)
    assert np.asarray(negatives).shape == (B, M, K, C)

    in_maps = make_in_maps(context, positive, negatives, mask_indices)
    res = run_bass_kernel_spmd(_get_nc(), in_maps, core_ids=list(range(B)))
    total = np.float64(0.0)
    for r in res.results:
        total += r["rowloss"].astype(np.float64).sum()
    return np.float32(total / (B * M))


# revision 16
# speedup vs baseline: 1.6447x; 1.0029x over previous
"""Trainium2 Bass kernel for nn_ContrastiveLoss (wav2vec2-style contrastive loss).

Shapes (hardcoded): B=8, C=256, T=1024, M=512 masked positions, K=100 negatives.
Sharding: pure data parallel - batch row b -> NeuronCore b (8 cores).

Strategy: negatives are uploaded as bf16 with each length-256 c-vector padded
to 258 (two zero pad slots). A runtime-registered custom DVE op streams a
[128, kch*258] tile once and emits, per element, an alternating pair of
running prefix sums: cumsum(neg*ctx) at even positions, cumsum(neg^2) at odd
positions (fp32 internal). The two pad slots at the end of each chunk hold the
chunk-complete cumsums of both quantities; a strided copy plus two subtracts
(on GpSimd, off the critical engine) recover per-k dots and sums of squares.
This computes BOTH reductions at ~1.008 DVE cycles per streamed element.

VectorE runs only the scan ops (+ tiny logit math); ScalarE does sqrt/exp/ln;
GpSimd does extraction/clamps. Small DMAs ride the ACT HWDGE queue so the SP
queue only carries the bulk negative stream. Group 0 starts with small tiles
so the first scan begins as early as possible; per-group epilogues are
interleaved so only group 3's tail is serialized. The device returns per-row
losses [128, 4] per core; the host sums and divides.
"""

import numpy as np

TEMP = 0.1
EPS = 1e-8
B, C, T = 8, 256, 1024
M = 512  # masked positions per batch row
K = 100  # negatives per masked position
P = 128  # partitions
G = M // P  # m-groups per core (4)
NPAD = C + 2  # padded chunk length (256 data + 2 pad)

# per-group tile splits (k0, kch); group 0 ramps up for an early first scan,
# growth rate matched to DMA vs scan speed (~1.25x)
KCH = 25  # k's per steady-state streamed tile
_R0 = [4, 5, 6, 8, 10, 12, 15, 20, 20]
_TILES0 = list(zip(np.cumsum([0] + _R0[:-1]).tolist(), _R0))
_TILESN = [(k0, KCH) for k0 in range(0, K, KCH)]
KCHMAX = max(KCH, max(_R0))

_NC = None
_OP = None

OP_NAME = "DUAL_CUMSUM_ANT"


def _register_op():
    """Register the dual-cumsum custom DVE op (idempotent)."""
    global _OP
    if _OP is not None:
        return _OP
    import concourse.dve_ops as dvo
    from concourse.dve_ops import DveOp
    from concourse.dve_spec import Spec, Src0, Src1, Zero, One, select, scan, AluOp, lower
    from concourse.dve_uop import DveOpSpec
    from concourse.dve_table_gen import dve_ver_for

    if OP_NAME in dvo._SUB_OPCODE_FOR_NAME:
        _OP = next(o for o in dvo.OPS if o.name == OP_NAME)
        return _OP

    def _ref(in0, in1, c0, c1, c2):
        Pp = in0.shape[0]
        a = np.asarray(in0, np.float32).reshape(Pp, -1)
        b = np.asarray(in1, np.float32).reshape(Pp, -1)
        prod = np.cumsum(a * b, axis=1)
        sqs = np.cumsum(a * a, axis=1)
        k = np.arange(a.shape[1])
        alt = (k % 2 == 0)  # xor-scan of ones seeded 0: TRUE at even positions
        return np.where(alt[None, :], prod, sqs).reshape(in0.shape)

    s1 = scan(AluOp.ADD, Src0 * Src1)
    s2 = scan(AluOp.ADD, Src0 * Src0)
    alt = scan(AluOp.LOGICAL_XOR, One, init=Zero)
    spec = Spec(body=select(alt, s1, s2), reference=_ref)

    row = max(dvo._SUB_OPCODE_FOR_NAME.values()) + 1
    assert row < 0x20
    dvo._SUB_OPCODE_FOR_NAME[OP_NAME] = row
    ver = dve_ver_for("TRN2")
    uops = lower(spec, ver=ver)
    sha = DveOpSpec(name=OP_NAME, opcode=row, uops=uops, rd1_en=True).sha(ver)
    op = DveOp(OP_NAME, spec, subdim=False, uops_sha={ver: sha})
    dvo.OPS.append(op)
    dvo.CUSTOM_DVE_SPECS[OP_NAME] = spec
    _OP = op
    return op


def _build_nc():
    import concourse.bacc as bacc
    import concourse.tile as tile
    from concourse import mybir

    op = _register_op()

    f32 = mybir.dt.float32
    bf16 = mybir.dt.bfloat16
    Alu = mybir.AluOpType
    Act = mybir.ActivationFunctionType
    EPS2 = EPS * EPS

    nc = bacc.Bacc(trn_type="TRN2")
    neg = nc.dram_tensor("neg", [M, K, NPAD], bf16, kind="ExternalInput")
    ctxp = nc.dram_tensor("ctxp", [M, NPAD], bf16, kind="ExternalInput")
    posg = nc.dram_tensor("posg", [M, C], bf16, kind="ExternalInput")
    rowloss = nc.dram_tensor("rowloss", [P, G], f32, kind="ExternalOutput")

    with tile.TileContext(nc) as tc:
        with (
            tc.tile_pool(name="stream", bufs=4) as stream,
            tc.tile_pool(name="bigp", bufs=3) as bigp,
            tc.tile_pool(name="grp", bufs=G) as grp,
            tc.tile_pool(name="pg", bufs=G) as pg,
            tc.tile_pool(name="scrp", bufs=2) as scrp,
            tc.tile_pool(name="outp", bufs=1) as outp,
        ):
            out_t = outp.tile([P, G], f32, tag="out_t")
            cps_t = outp.tile([P, 2 * G], f32, tag="cps_t")  # css col g, pss col G+g
            cpd_t = outp.tile([P, G], f32, tag="cpd_t")
            l0_t = outp.tile([P, G], f32, tag="l0_t")
            mx_t = outp.tile([P, G], f32, tag="mx_t")
            mxs_t = outp.tile([P, G], f32, tag="mxs_t")
            se_t = outp.tile([P, G], f32, tag="se_t")
            lnse_t = outp.tile([P, G], f32, tag="lnse_t")
            t1_t = outp.tile([P, G], f32, tag="t1_t")
            cumt0 = outp.tile([P, KCHMAX + 1, 2], f32, tag="cumt0")
            cumt1 = outp.tile([P, KCHMAX + 1, 2], f32, tag="cumt1")
            cumts = [cumt0, cumt1]
            nc.gpsimd.memset(cumt0[:, 0:1, :], 0.0)
            nc.gpsimd.memset(cumt1[:, 0:1, :], 0.0)

            gt = {}
            for g in range(G):
                gt[g] = dict(
                    rawdots=pg.tile([P, K], f32, tag="rawdots", name=f"rawdots{g}"),
                    negss=pg.tile([P, K], f32, tag="negss", name=f"negss{g}"),
                    logits=pg.tile([P, K + 1], f32, tag="logits", name=f"logits{g}"),
                    nrn=pg.tile([P, K], f32, tag="nrn", name=f"nrn{g}"),
                )

            # --- up-front small DMAs (ACT HWDGE queue) + ScalarE prologue ---
            ctx_ts, pos_ts = {}, {}
            for g in range(G):
                m0 = g * P
                ctx_ts[g] = grp.tile([P, NPAD], bf16, tag="ctx", name=f"ctx{g}")
                nc.scalar.dma_start(out=ctx_ts[g][:], in_=ctxp[m0 : m0 + P, :])
                pos_ts[g] = grp.tile([P, C], bf16, tag="pos", name=f"pos{g}")
                nc.scalar.dma_start(out=pos_ts[g][:], in_=posg[m0 : m0 + P, :])
            for g in range(G):
                sq_s = scrp.tile([P, C], f32, tag="sq_s")
                nc.scalar.activation(
                    out=sq_s[:], in_=ctx_ts[g][:, 0:C], func=Act.Square,
                    accum_out=cps_t[:, g : g + 1],
                )
                sq_s2 = scrp.tile([P, C], f32, tag="sq_s2")
                nc.scalar.activation(
                    out=sq_s2[:], in_=pos_ts[g][:], func=Act.Square,
                    accum_out=cps_t[:, G + g : G + g + 1],
                )

            def prologue_dve():
                # issued after the first scans so it never delays scan 0;
                # results are first needed by epilogue(0), much later
                for g in range(G):
                    scr = scrp.tile([P, C], bf16, tag="scr")
                    nc.vector.scalar_tensor_tensor(
                        out=scr[:], in0=ctx_ts[g][:, 0:C], scalar=1.0,
                        in1=pos_ts[g][:], op0=Alu.mult, op1=Alu.mult,
                        accum_out=cpd_t[:, g : g + 1],
                    )
                # crn/prn = 1/max(sqrt(ss), EPS) = 1/sqrt(max(ss, EPS^2))
                nc.vector.tensor_scalar_max(cps_t[:], cps_t[:], EPS2)
                nc.scalar.sqrt(cps_t[:], cps_t[:])
                nc.vector.reciprocal_approx_fast(cps_t[:], cps_t[:])

            def epilogue(g):
                d = gt[g]
                nc.vector.tensor_scalar_max(d["negss"][:], d["negss"][:], EPS2)
                nc.scalar.sqrt(d["negss"][:], d["negss"][:])
                nc.vector.reciprocal_approx_fast(d["nrn"][:], d["negss"][:])
                nc.vector.scalar_tensor_tensor(
                    out=d["logits"][:, 0:1], in0=cpd_t[:, g : g + 1],
                    scalar=cps_t[:, g : g + 1], in1=cps_t[:, G + g : G + g + 1],
                    op0=Alu.mult, op1=Alu.mult,
                )
                nc.vector.tensor_copy(l0_t[:, g : g + 1], d["logits"][:, 0:1])
                nc.vector.scalar_tensor_tensor(
                    out=d["logits"][:, 1 : K + 1], in0=d["rawdots"][:],
                    scalar=cps_t[:, g : g + 1], in1=d["nrn"][:],
                    op0=Alu.mult, op1=Alu.mult,
                )
                nc.vector.reduce_max(
                    mx_t[:, g : g + 1], d["logits"][:], axis=mybir.AxisListType.X
                )
                nc.vector.tensor_scalar_mul(
                    mxs_t[:, g : g + 1], mx_t[:, g : g + 1], -1.0 / TEMP
                )
                esc = scrp.tile([P, K + 1], f32, tag="esc")
                nc.scalar.activation(
                    out=esc[:], in_=d["logits"][:], func=Act.Exp,
                    scale=1.0 / TEMP, bias=mxs_t[:, g : g + 1],
                    accum_out=se_t[:, g : g + 1],
                )
                if g == G - 2:
                    # prewarm the Sqrt ACT table so group G-1's tail chain
                    # skips one table load
                    dum = scrp.tile([P, 1], f32, tag="dum")
                    nc.scalar.sqrt(dum[:], se_t[:, 0:1])

            # --- main streaming loop ---
            # subs for tile t are issued after scan t+1 so the ACT-side
            # extraction copy overlaps the next scan instead of stalling DVE;
            # epilogue(g-1) is deferred into group g
            pending_subs = []
            tile_no = 0

            def flush_subs():
                while pending_subs:
                    pending_subs.pop(0)()

            for g in range(G):
                m0 = g * P
                d = gt[g]
                ctx_t = ctx_ts[g]
                tiles = _TILES0 if g == 0 else _TILESN
                for i, (k0, kch) in enumerate(tiles):
                    nt = stream.tile([P, kch, NPAD], bf16, tag="nt")
                    nc.sync.dma_start(
                        out=nt[:], in_=neg[m0 : m0 + P, k0 : k0 + kch, :]
                    )
                    big = bigp.tile([P, kch * NPAD], f32, tag="big")
                    ctx_bc = ctx_t[:].unsqueeze(1).broadcast_to([P, kch, NPAD])
                    nc.vector._custom_dve(
                        op, out=big[:], in0=nt[:], in1=ctx_bc, s0=0.0, s1=0.0
                    )
                    big3 = big[:].rearrange("p (s n) -> p s n", s=kch)
                    cumt = cumts[tile_no % 2]
                    tile_no += 1
                    nc.scalar.copy(cumt[:, 1 : kch + 1, :], big3[:, :, C : C + 2])

                    def subs(d=d, k0=k0, kch=kch, cumt=cumt):
                        # dot cumsum at even pad slot (C), sq cumsum at C+1
                        nc.vector.tensor_sub(
                            d["rawdots"][:, k0 : k0 + kch],
                            cumt[:, 1 : kch + 1, 0],
                            cumt[:, 0:kch, 0],
                        )
                        nc.vector.tensor_sub(
                            d["negss"][:, k0 : k0 + kch],
                            cumt[:, 1 : kch + 1, 1],
                            cumt[:, 0:kch, 1],
                        )

                    flush_subs()
                    pending_subs.append(subs)
                    if g == 0 and i == 2:
                        prologue_dve()
                    if i == 1 and g > 0:
                        epilogue(g - 1)
            flush_subs()
            epilogue(G - 1)

            # --- tail ---
            nc.scalar.activation(out=lnse_t[:], in_=se_t[:], func=Act.Ln)
            nc.vector.scalar_tensor_tensor(
                out=t1_t[:], in0=mx_t[:], scalar=1.0 / TEMP, in1=lnse_t[:],
                op0=Alu.mult, op1=Alu.add,
            )
            nc.vector.scalar_tensor_tensor(
                out=out_t[:], in0=l0_t[:], scalar=-1.0 / TEMP, in1=t1_t[:],
                op0=Alu.mult, op1=Alu.add,
            )
            nc.scalar.dma_start(out=rowloss[:], in_=out_t[:])
    nc.finalize()
    return nc


def _get_nc():
    global _NC
    if _NC is None:
        _NC = _build_nc()
    return _NC


def make_in_maps(context, positive, negatives, mask_indices):
    import ml_dtypes

    bf = ml_dtypes.bfloat16
    context = np.asarray(context, dtype=np.float32)
    positive = np.asarray(positive, dtype=np.float32)
    negatives = np.asarray(negatives, dtype=np.float32)
    mask = np.asarray(mask_indices).astype(bool)

    in_maps = []
    for b in range(B):
        idx = np.flatnonzero(mask[b])
        assert idx.size == M, f"row {b}: expected {M} masked, got {idx.size}"
        ctxg = context[b].T[idx]  # [M, C] f32
        posg = positive[b].T[idx]
        ctxp = np.zeros((M, NPAD), dtype=bf)
        ctxp[:, :C] = ctxg.astype(bf)
        negp = np.zeros((M, K, NPAD), dtype=bf)
        negp[:, :, :C] = negatives[b].astype(bf)
        in_maps.append(
            {
                "neg": negp,
                "ctxp": ctxp,
                "posg": np.ascontiguousarray(posg.astype(bf)),
            }
        )
    return in_maps


def kernel(context, positive, negatives, mask_indices, num_masked):
    from concourse.bass_utils import run_bass_kernel_spmd

    nm = int(np.asarray(num_masked))
    assert nm == M, f"kernel hardcodes num_masked={M}, got {nm}"
    assert np.asarray(context).shape == (B, C, T)
    assert np.asarray(negatives).shape == (B, M, K, C)

    in_maps = make_in_maps(context, positive, negatives, mask_indices)
    res = run_bass_kernel_spmd(_get_nc(), in_maps, core_ids=list(range(B)))
    total = np.float64(0.0)
    for r in res.results:
        total += r["rowloss"].astype(np.float64).sum()
    return np.float32(total / (B * M))


# revision 22
# speedup vs baseline: 1.6698x; 1.0153x over previous
"""Trainium2 Bass kernel for nn_ContrastiveLoss (wav2vec2-style contrastive loss).

Shapes (hardcoded): B=8, C=256, T=1024, M=512 masked positions, K=100 negatives.
Sharding: pure data parallel - batch row b -> NeuronCore b (8 cores).

Strategy: negatives are uploaded as bf16 with each length-256 c-vector padded
to 258 (two zero pad slots). A runtime-registered custom DVE op streams a
[128, kch*258] tile once and emits, per element, an alternating pair of
running prefix sums: cumsum(neg*ctx) at even positions, cumsum(neg^2) at odd
positions (fp32 internal). The two pad slots at the end of each chunk hold the
chunk-complete cumsums of both quantities; a strided copy plus two subtracts
(on GpSimd, off the critical engine) recover per-k dots and sums of squares.
This computes BOTH reductions at ~1.008 DVE cycles per streamed element.

VectorE runs only the scan ops (+ tiny logit math); ScalarE does sqrt/exp/ln;
GpSimd does extraction/clamps. Small DMAs ride the ACT HWDGE queue so the SP
queue only carries the bulk negative stream. Group 0 starts with small tiles
so the first scan begins as early as possible; per-group epilogues are
interleaved so only group 3's tail is serialized. The device returns per-row
losses [128, 4] per core; the host sums and divides.
"""

import numpy as np

TEMP = 0.1
EPS = 1e-8
B, C, T = 8, 256, 1024
M = 512  # masked positions per batch row
K = 100  # negatives per masked position
P = 128  # partitions
G = M // P  # m-groups per core (4)
NPAD = C + 2  # padded chunk length (256 data + 2 pad)

# per-group tile splits (k0, kch); group 0 ramps up for an early first scan,
# growth rate matched to DMA vs scan speed (~1.25x)
KCH = 25  # k's per steady-state streamed tile
_R0 = [4, 5, 6, 8, 10, 12, 15, 20, 20]
_TILES0 = list(zip(np.cumsum([0] + _R0[:-1]).tolist(), _R0))
_TILESN = [(k0, KCH) for k0 in range(0, K, KCH)]
KCHMAX = max(KCH, max(_R0))

_NC = None
_OP = None

OP_NAME = "DUAL_CUMSUM_ANT"


def _register_op():
    """Register the dual-cumsum custom DVE op (idempotent)."""
    global _OP
    if _OP is not None:
        return _OP
    import concourse.dve_ops as dvo
    from concourse.dve_ops import DveOp
    from concourse.dve_spec import Spec, Src0, Src1, Zero, One, select, scan, AluOp, lower
    from concourse.dve_uop import DveOpSpec
    from concourse.dve_table_gen import dve_ver_for

    if OP_NAME in dvo._SUB_OPCODE_FOR_NAME:
        _OP = next(o for o in dvo.OPS if o.name == OP_NAME)
        return _OP

    def _ref(in0, in1, c0, c1, c2):
        Pp = in0.shape[0]
        a = np.asarray(in0, np.float32).reshape(Pp, -1)
        b = np.asarray(in1, np.float32).reshape(Pp, -1)
        prod = np.cumsum(a * b, axis=1)
        sqs = np.cumsum(a * a, axis=1)
        k = np.arange(a.shape[1])
        alt = (k % 2 == 0)  # xor-scan of ones seeded 0: TRUE at even positions
        return np.where(alt[None, :], prod, sqs).reshape(in0.shape)

    s1 = scan(AluOp.ADD, Src0 * Src1)
    s2 = scan(AluOp.ADD, Src0 * Src0)
    alt = scan(AluOp.LOGICAL_XOR, One, init=Zero)
    spec = Spec(body=select(alt, s1, s2), reference=_ref)

    row = max(dvo._SUB_OPCODE_FOR_NAME.values()) + 1
    assert row < 0x20
    dvo._SUB_OPCODE_FOR_NAME[OP_NAME] = row
    ver = dve_ver_for("TRN2")
    uops = lower(spec, ver=ver)
    sha = DveOpSpec(name=OP_NAME, opcode=row, uops=uops, rd1_en=True).sha(ver)
    op = DveOp(OP_NAME, spec, subdim=False, uops_sha={ver: sha})
    dvo.OPS.append(op)
    dvo.CUSTOM_DVE_SPECS[OP_NAME] = spec
    _OP = op
    return op


def _build_nc():
    import concourse.bacc as bacc
    import concourse.tile as tile
    from concourse import mybir

    op = _register_op()

    f32 = mybir.dt.float32
    bf16 = mybir.dt.bfloat16
    Alu = mybir.AluOpType
    Act = mybir.ActivationFunctionType
    EPS2 = EPS * EPS

    nc = bacc.Bacc(trn_type="TRN2")
    neg = nc.dram_tensor("neg", [M, K, NPAD], bf16, kind="ExternalInput")
    ctxp = nc.dram_tensor("ctxp", [M, NPAD], bf16, kind="ExternalInput")
    posg = nc.dram_tensor("posg", [M, C], bf16, kind="ExternalInput")
    rowloss = nc.dram_tensor("rowloss", [P, G], f32, kind="ExternalOutput")

    with tile.TileContext(nc) as tc:
        with (
            tc.tile_pool(name="stream", bufs=4) as stream,
            tc.tile_pool(name="bigp", bufs=3) as bigp,
            tc.tile_pool(name="grp", bufs=G) as grp,
            tc.tile_pool(name="pg", bufs=G) as pg,
            tc.tile_pool(name="scrp", bufs=2) as scrp,
            tc.tile_pool(name="outp", bufs=1) as outp,
        ):
            out_t = outp.tile([P, G], f32, tag="out_t")
            cps_t = outp.tile([P, 2 * G], f32, tag="cps_t")  # css col g, pss col G+g
            cpd_t = outp.tile([P, G], f32, tag="cpd_t")
            l0_t = outp.tile([P, G], f32, tag="l0_t")
            se_t = outp.tile([P, G], f32, tag="se_t")
            lnse_t = outp.tile([P, G], f32, tag="lnse_t")
            t1_t = outp.tile([P, G], f32, tag="t1_t")
            cumt0 = outp.tile([P, KCHMAX + 1, 2], f32, tag="cumt0")
            cumt1 = outp.tile([P, KCHMAX + 1, 2], f32, tag="cumt1")
            cumts = [cumt0, cumt1]
            nc.gpsimd.memset(cumt0[:, 0:1, :], 0.0)
            nc.gpsimd.memset(cumt1[:, 0:1, :], 0.0)
            biasc = outp.tile([P, 1], f32, tag="biasc")
            nc.gpsimd.memset(biasc[:], -1.0 / TEMP)

            gt = {}
            for g in range(G):
                gt[g] = dict(
                    rawdots=pg.tile([P, K], f32, tag="rawdots", name=f"rawdots{g}"),
                    negss=pg.tile([P, K], f32, tag="negss", name=f"negss{g}"),
                    logits=pg.tile([P, K + 1], f32, tag="logits", name=f"logits{g}"),
                    nrn=pg.tile([P, K], f32, tag="nrn", name=f"nrn{g}"),
                )

            # --- up-front small DMAs (ACT HWDGE queue) + ScalarE prologue ---
            ctx_ts, pos_ts = {}, {}
            for g in range(G):
                m0 = g * P
                ctx_ts[g] = grp.tile([P, NPAD], bf16, tag="ctx", name=f"ctx{g}")
                # group 0's ctx rides the SP queue ahead of the first neg
                # tile so the first scan's inputs land as early as possible
                eng = nc.sync if g == 0 else nc.scalar
                eng.dma_start(out=ctx_ts[g][:], in_=ctxp[m0 : m0 + P, :])
                pos_ts[g] = grp.tile([P, C], bf16, tag="pos", name=f"pos{g}")
                nc.scalar.dma_start(out=pos_ts[g][:], in_=posg[m0 : m0 + P, :])
            for g in range(G):
                sq_s = scrp.tile([P, C], f32, tag="sq_s")
                nc.scalar.activation(
                    out=sq_s[:], in_=ctx_ts[g][:, 0:C], func=Act.Square,
                    accum_out=cps_t[:, g : g + 1],
                )
                sq_s2 = scrp.tile([P, C], f32, tag="sq_s2")
                nc.scalar.activation(
                    out=sq_s2[:], in_=pos_ts[g][:], func=Act.Square,
                    accum_out=cps_t[:, G + g : G + g + 1],
                )

            def prologue_dve():
                # issued after the first scans so it never delays scan 0;
                # results are first needed by epilogue(0), much later
                for g in range(G):
                    scr = scrp.tile([P, C], bf16, tag="scr")
                    nc.vector.scalar_tensor_tensor(
                        out=scr[:], in0=ctx_ts[g][:, 0:C], scalar=1.0,
                        in1=pos_ts[g][:], op0=Alu.mult, op1=Alu.mult,
                        accum_out=cpd_t[:, g : g + 1],
                    )
                # crn/prn = 1/max(sqrt(ss), EPS) = 1/sqrt(max(ss, EPS^2))
                nc.vector.tensor_scalar_max(cps_t[:], cps_t[:], EPS2)
                nc.scalar.sqrt(cps_t[:], cps_t[:])
                nc.vector.reciprocal_approx_fast(cps_t[:], cps_t[:])

            def epilogue(g):
                d = gt[g]
                nc.vector.tensor_scalar_max(d["negss"][:], d["negss"][:], EPS2)
                nc.scalar.sqrt(d["negss"][:], d["negss"][:])
                if g == G - 1:
                    # prewarm the Exp ACT table (runs while DVE computes the
                    # logits below) so the tail exp skips its table load
                    dum2 = scrp.tile([P, 1], f32, tag="dum2")
                    nc.scalar.activation(out=dum2[:], in_=se_t[:, 0:1], func=Act.Exp)
                nc.vector.reciprocal_approx_fast(d["nrn"][:], d["negss"][:])
                nc.vector.scalar_tensor_tensor(
                    out=d["logits"][:, 0:1], in0=cpd_t[:, g : g + 1],
                    scalar=cps_t[:, g : g + 1], in1=cps_t[:, G + g : G + g + 1],
                    op0=Alu.mult, op1=Alu.mult,
                )
                nc.vector.tensor_copy(l0_t[:, g : g + 1], d["logits"][:, 0:1])
                nc.vector.scalar_tensor_tensor(
                    out=d["logits"][:, 1 : K + 1], in0=d["rawdots"][:],
                    scalar=cps_t[:, g : g + 1], in1=d["nrn"][:],
                    op0=Alu.mult, op1=Alu.mult,
                )
                # |cosine| <= 1 so logits/TEMP <= 10: a constant shift of -10
                # replaces the per-row max (logsumexp is shift-invariant)
                esc = scrp.tile([P, K + 1], f32, tag="esc")
                nc.scalar.activation(
                    out=esc[:], in_=d["logits"][:], func=Act.Exp,
                    scale=1.0 / TEMP, bias=biasc[:],
                    accum_out=se_t[:, g : g + 1],
                )
                if g == G - 2:
                    # prewarm the Sqrt ACT table so group G-1's tail chain
                    # skips one table load
                    dum = scrp.tile([P, 1], f32, tag="dum")
                    nc.scalar.sqrt(dum[:], se_t[:, 0:1])

            # --- main streaming loop ---
            # subs for tile t are issued after scan t+1 so the ACT-side
            # extraction copy overlaps the next scan instead of stalling DVE;
            # epilogue(g-1) is deferred into group g
            pending_subs = []
            tile_no = 0

            def flush_subs():
                while pending_subs:
                    pending_subs.pop(0)()

            for g in range(G):
                m0 = g * P
                d = gt[g]
                ctx_t = ctx_ts[g]
                tiles = _TILES0 if g == 0 else _TILESN
                for i, (k0, kch) in enumerate(tiles):
                    nt = stream.tile([P, kch, NPAD], bf16, tag="nt")
                    nc.sync.dma_start(
                        out=nt[:], in_=neg[m0 : m0 + P, k0 : k0 + kch, :]
                    )
                    big = bigp.tile([P, kch * NPAD], f32, tag="big")
                    ctx_bc = ctx_t[:].unsqueeze(1).broadcast_to([P, kch, NPAD])
                    nc.vector._custom_dve(
                        op, out=big[:], in0=nt[:], in1=ctx_bc, s0=0.0, s1=0.0
                    )
                    big3 = big[:].rearrange("p (s n) -> p s n", s=kch)
                    cumt = cumts[tile_no % 2]
                    tile_no += 1
                    nc.scalar.copy(cumt[:, 1 : kch + 1, :], big3[:, :, C : C + 2])

                    def subs(d=d, k0=k0, kch=kch, cumt=cumt):
                        # dot cumsum at even pad slot (C), sq cumsum at C+1
                        nc.vector.tensor_sub(
                            d["rawdots"][:, k0 : k0 + kch],
                            cumt[:, 1 : kch + 1, 0],
                            cumt[:, 0:kch, 0],
                        )
                        nc.vector.tensor_sub(
                            d["negss"][:, k0 : k0 + kch],
                            cumt[:, 1 : kch + 1, 1],
                            cumt[:, 0:kch, 1],
                        )

                    flush_subs()
                    pending_subs.append(subs)
                    if g == 0 and i == 2:
                        prologue_dve()
                    if i == 1 and g > 0:
                        epilogue(g - 1)
            flush_subs()
            epilogue(G - 1)

            # --- tail: loss = (1/TEMP + ln(se)) - l0/TEMP ---
            nc.scalar.activation(out=lnse_t[:], in_=se_t[:], func=Act.Ln)
            nc.vector.tensor_scalar_add(t1_t[:], lnse_t[:], 1.0 / TEMP)
            nc.vector.scalar_tensor_tensor(
                out=out_t[:], in0=l0_t[:], scalar=-1.0 / TEMP, in1=t1_t[:],
                op0=Alu.mult, op1=Alu.add,
            )
            nc.scalar.dma_start(out=rowloss[:], in_=out_t[:])
    nc.finalize()
    return nc


def _get_nc():
    global _NC
    if _NC is None:
        _NC = _build_nc()
    return _NC


def make_in_maps(context, positive, negatives, mask_indices):
    import ml_dtypes

    bf = ml_dtypes.bfloat16
    context = np.asarray(context, dtype=np.float32)
    positive = np.asarray(positive, dtype=np.float32)
    negatives = np.asarray(negatives, dtype=np.float32)
    mask = np.asarray(mask_indices).astype(bool)

    in_maps = []
    for b in range(B):
        idx = np.flatnonzero(mask[b])
        assert idx.size == M, f"row {b}: expected {M} masked, got {idx.size}"
        ctxg = context[b].T[idx]  # [M, C] f32
        posg = positive[b].T[idx]
        ctxp = np.zeros((M, NPAD), dtype=bf)
        ctxp[:, :C] = ctxg.astype(bf)
        negp = np.zeros((M, K, NPAD), dtype=bf)
        negp[:, :, :C] = negatives[b].astype(bf)
        in_maps.append(
            {
                "neg": negp,
                "ctxp": ctxp,
                "posg": np.ascontiguousarray(posg.astype(bf)),
            }
        )
    return in_maps


def kernel(context, positive, negatives, mask_indices, num_masked):
    from concourse.bass_utils import run_bass_kernel_spmd

    nm = int(np.asarray(num_masked))
    assert nm == M, f"kernel hardcodes num_masked={M}, got {nm}"
    assert np.asarray(context).shape == (B, C, T)
    assert np.asarray(negatives).shape == (B, M, K, C)

    in_maps = make_in_maps(context, positive, negatives, mask_indices)
    res = run_bass_kernel_spmd(_get_nc(), in_maps, core_ids=list(range(B)))
    total = np.float64(0.0)
    for r in res.results:
        total += r["rowloss"].astype(np.float64).sum()
    return np.float32(total / (B * M))
